# revision 3
# baseline (speedup 1.0000x reference)
"""Multi-head attention block (QKV proj + masked softmax attention + dense +
residual + LayerNorm) on 8 NeuronCores.

Sharding: tensor-parallel over heads for attention (2 heads / core), then an
AllToAll exchange, then data-parallel over batch for dense+residual+LN.

Self-contained: includes the TileContext drain patch and the multi-wait BIR
split pass required by the walrus build in this environment.
"""
import functools

import numpy as np

import concourse.bass as bass
import concourse.mybir as mybir
import concourse.tile as tile
from concourse import bass_utils
from concourse.vector_clock import ScopedClock, VectorClock

F32 = mybir.dt.float32
BF16 = mybir.dt.bfloat16
I32 = mybir.dt.int32
AF = mybir.ActivationFunctionType
ALU = mybir.AluOpType

B, S, IN_DIM, D_MODEL, H, DEPTH = 8, 1024, 512, 1024, 16, 64
NCORES = 8
COLS = D_MODEL // NCORES  # 128 columns of D_MODEL per core (2 heads)
TOK = B * S  # 8192
SCALE = 1.0 / 8.0
LN_EPS = 1e-5


# ---------------------------------------------------------------------------
# Environment workarounds
# ---------------------------------------------------------------------------
class _PatchedTileContext(tile.TileContext):
    """walrus here rejects >1 sem-wait on CTRL (Drain/NoOp) instructions; the
    TileContext exit drain carries one wait per live proc. Emit one NOP per
    proc wait instead (add_sem_waits elides already-covered procs)."""

    def _drain_and_barrier(self, tick_clock, wait_clock):
        gc = tick_clock.global_clock
        scoped = ScopedClock({None: gc})
        for scope, vclock in scoped.items():
            ticks = eval(repr(vclock)[len("VectorClock(") : -1])
            for proc, tick_v in enumerate(ticks):
                if tick_v <= 0:
                    continue
                v = VectorClock()
                v.require_at_least(proc, tick_v)
                nop_inst = self.nc.sync.nop(nofuse=True, hint=f"drain_wait_p{proc}")
                wait_clock.add_sem_waits(nop_inst.ins, ScopedClock({scope: v}))
        self.nc.sync.drain()
        self.nc.all_engine_barrier()
        assert self.sems is not None
        popped = self.nc._tile_sem_poison_stack.pop()
        assert popped is self._sem_poison
        self.nc.clear_and_free_semaphores(list(self.sems.allocated().values()))
        self.nc.all_engine_barrier()


def _clone_wait(w):
    return mybir.SyncWait(
        sync_type=w.sync_type, id=w.id, ant_name=w.ant_name,
        wait_mode=w.wait_mode, wait_value=w.wait_value, wait_reg=w.wait_reg,
    )


def _split_multi_waits(nc):
    """walrus allows one sem-wait per instruction; hoist extras onto preceding
    same-engine NoOps (sequencers execute in order, so this is equivalent)."""
    for f in nc.m.functions:
        for blk in f.blocks:
            out = []
            for ins in blk.instructions:
                si = ins.sync_info
                if si is not None and si.on_wait is not None and len(si.on_wait) > 1:
                    waits = [_clone_wait(w) for w in si.on_wait]
                    for k, w in enumerate(waits[:-1]):
                        nop = mybir.InstNoOp(
                            name=f"{ins.name}_sw{k}",
                            sync_info=mybir.SyncInfo(on_wait=[w], on_update=[]),
                        )
                        nop.engine = ins.engine
                        out.append(nop)
                    ins.sync_info = mybir.SyncInfo(
                        on_wait=[waits[-1]],
                        on_update=list(si.on_update) if si.on_update else [],
                    )
                out.append(ins)
            try:
                blk.instructions = out
            except Exception:
                cur = blk.instructions
                cur.clear()
                cur.extend(out)


def _bcast_ap(ap_1d, nparts):
    """AP reading a 1-D DRAM tensor replicated across nparts partitions."""
    return bass.AP(
        tensor=ap_1d.tensor, offset=ap_1d.offset,
        ap=[[0, nparts]] + list(ap_1d.ap),
    )


# ---------------------------------------------------------------------------
# Kernel build
# ---------------------------------------------------------------------------
@functools.lru_cache(maxsize=1)
def _build():
    nc = bass.Bass("TRN2", target_bir_lowering=False, debug=False,
                   num_devices=NCORES)

    xT = nc.dram_tensor("xT", [IN_DIM, TOK], F32, kind="ExternalInput")
    xTres = nc.dram_tensor("xTres", [IN_DIM, S], F32, kind="ExternalInput")
    maskT = nc.dram_tensor("maskT", [2, S, S], I32, kind="ExternalInput")
    wq = nc.dram_tensor("wq", [IN_DIM, COLS], F32, kind="ExternalInput")
    wk = nc.dram_tensor("wk", [IN_DIM, COLS], F32, kind="ExternalInput")
    wv = nc.dram_tensor("wv", [IN_DIM, COLS], F32, kind="ExternalInput")
    qb = nc.dram_tensor("qb", [COLS], F32, kind="ExternalInput")
    kb = nc.dram_tensor("kb", [COLS], F32, kind="ExternalInput")
    vb = nc.dram_tensor("vb", [COLS], F32, kind="ExternalInput")
    dw = nc.dram_tensor("dw", [D_MODEL, D_MODEL], F32, kind="ExternalInput")
    db = nc.dram_tensor("db", [D_MODEL], F32, kind="ExternalInput")
    rw = nc.dram_tensor("rw", [IN_DIM, D_MODEL], F32, kind="ExternalInput")
    lng = nc.dram_tensor("lng", [D_MODEL], F32, kind="ExternalInput")
    lnb = nc.dram_tensor("lnb", [D_MODEL], F32, kind="ExternalInput")
    out = nc.dram_tensor("out", [S, D_MODEL], F32, kind="ExternalOutput")

    # collective buffers (bf16): [batch, this-core cols, tokens-of-batch]
    cc_in = nc.dram_tensor("cc_in", [B, COLS, S], BF16)
    cc_out = nc.dram_tensor("cc_out", [NCORES, COLS, S], BF16)

    KT_IN = IN_DIM // 128   # 4 contraction tiles for IN_DIM
    KT_D = D_MODEL // 128   # 8 contraction tiles for D_MODEL
    NTT = S // 128          # 8 token tiles per batch

    with _PatchedTileContext(nc) as tc:
        with (
            tc.tile_pool(name="persist", bufs=1) as persist,
        ):
            # ---------------- weights / constants ----------------
            wq_bf = persist.tile([128, KT_IN, COLS], BF16, tag="wq")
            wk_bf = persist.tile([128, KT_IN, COLS], BF16, tag="wk")
            wv_bf = persist.tile([128, KT_IN, COLS], BF16, tag="wv")
            for wt, wsrc in ((wq_bf, wq), (wk_bf, wk), (wv_bf, wv)):
                nc.gpsimd.dma_start(
                    out=wt[:], in_=wsrc.ap().rearrange("(kt p) c -> p kt c", p=128))
            qb_sb = persist.tile([128, 1], F32, tag="qb")
            kb_sb = persist.tile([128, 1], F32, tag="kb")
            nc.sync.dma_start(out=qb_sb[:], in_=qb.ap())
            nc.sync.dma_start(out=kb_sb[:], in_=kb.ap())
            vb_bc = persist.tile([128, COLS], F32, tag="vbc")
            nc.gpsimd.dma_start(out=vb_bc[:], in_=_bcast_ap(vb.ap(), 128))
            eps_sb = persist.tile([128, 1], F32, tag="eps")
            nc.vector.memset(eps_sb[:], LN_EPS)
            ones_s = persist.tile([1, 64], F32, tag="ones")
            nc.vector.memset(ones_s[:], 1.0)

            # residual x slice (this batch), bf16 [128, kt, S]
            xres_bf = persist.tile([128, KT_IN, S], BF16, tag="xres")
            nc.gpsimd.dma_start(
                out=xres_bf[:],
                in_=xTres.ap().rearrange("(kt p) t -> p kt t", p=128))

            # q/k (depth-major) and v-augmented (token-major) for all batches
            q_sb = persist.tile([128, TOK], BF16, tag="qsb")
            k_sb = persist.tile([128, TOK], BF16, tag="ksb")
            v_aug = persist.tile([128, B * NTT, 2, 65], BF16, tag="vaug")
            nc.vector.memset(v_aug[:, :, :, 64:65], 1.0)

            # ---------------- phase A: QKV projections ----------------
            with (
                tc.tile_pool(name="xbf_pool", bufs=2) as xbf_pool,
                tc.tile_pool(name="ps_qkv", bufs=4, space="PSUM") as ps_qkv,
            ):
                for b in range(B):
                    bsl = slice(b * S, (b + 1) * S)
                    x_b = xbf_pool.tile([128, KT_IN, S], BF16, tag="xb")
                    nc.gpsimd.dma_start(
                        out=x_b[:],
                        in_=xT.ap()[:, bsl].rearrange("(kt p) t -> p kt t", p=128))

                    # q, k: out_T [cols, tok] — weights stationary, x moving
                    for tchunk in range(2):
                        tsl = slice(tchunk * 512, (tchunk + 1) * 512)
                        gsl = slice(b * S + tchunk * 512, b * S + (tchunk + 1) * 512)
                        for wt, bias, dst in ((wq_bf, qb_sb, q_sb),
                                              (wk_bf, kb_sb, k_sb)):
                            p = ps_qkv.tile([128, 512], F32, tag="pqk")
                            for kt in range(KT_IN):
                                nc.tensor.matmul(
                                    p[:], lhsT=wt[:, kt, :], rhs=x_b[:, kt, tsl],
                                    start=(kt == 0), stop=(kt == KT_IN - 1))
                            nc.vector.tensor_scalar_add(
                                out=dst[:, gsl], in0=p[:], scalar1=bias[:])

                    # v: token-major [tok, cols] — x stationary, wv moving
                    for tt in range(NTT):
                        p = ps_qkv.tile([128, COLS], F32, tag="pv")
                        gtt = b * NTT + tt
                        tsl = slice(tt * 128, (tt + 1) * 128)
                        for kt in range(KT_IN):
                            nc.tensor.matmul(
                                p[:], lhsT=x_b[:, kt, tsl], rhs=wv_bf[:, kt, :],
                                start=(kt == 0), stop=(kt == KT_IN - 1))
                        for hh in range(2):
                            csl = slice(hh * 64, hh * 64 + 64)
                            nc.vector.tensor_add(
                                out=v_aug[:, gtt, hh, 0:64],
                                in0=p[:, csl], in1=vb_bc[:, csl])

            # ---------------- phase B: attention per (batch, head) ----------
            with (
                tc.tile_pool(name="mpool", bufs=1) as mpool,
                tc.tile_pool(name="pt", bufs=2) as ptp,
                tc.tile_pool(name="stage", bufs=2) as stagep,
                tc.tile_pool(name="ps_s", bufs=3, space="PSUM") as ps_s,
                tc.tile_pool(name="ps_av", bufs=1, space="PSUM") as ps_av,
                tc.tile_pool(name="ps_inv", bufs=1, space="PSUM") as ps_inv,
            ):
                m_bf = mpool.tile([128, 2, NTT, S], BF16, tag="mbf")
                nc.gpsimd.dma_start(
                    out=m_bf[:],
                    in_=maskT.ap().rearrange("h (kt p) q -> p h kt q", p=128))
                for b in range(B):
                    for hh in range(2):
                        rsl = slice(hh * 64, hh * 64 + 64)
                        p_t = ptp.tile([128, NTT, S], BF16, tag="pt")
                        for kt in range(NTT):
                            ksl = slice(b * S + kt * 128, b * S + (kt + 1) * 128)
                            for qc in range(2):
                                qsl = slice(b * S + qc * 512, b * S + (qc + 1) * 512)
                                ps = ps_s.tile([128, 512], F32, tag="ps")
                                nc.tensor.matmul(
                                    ps[:], lhsT=k_sb[rsl, ksl], rhs=q_sb[rsl, qsl],
                                    start=True, stop=True,
                                    tile_position=(hh * 64, 0))
                                nc.scalar.activation(
                                    out=p_t[:, kt, qc * 512:(qc + 1) * 512],
                                    in_=ps[:], func=AF.Exp, scale=SCALE)
                            nc.vector.tensor_mul(
                                out=p_t[:, kt, :], in0=p_t[:, kt, :],
                                in1=m_bf[:, hh, kt, :])

                        # AV + sums: V augmented with a ones column (row 64)
                        po = ps_av.tile([65, S], F32, tag="po")
                        for kt in range(NTT):
                            for qc in range(2):
                                nc.tensor.matmul(
                                    po[:, qc * 512:(qc + 1) * 512],
                                    lhsT=v_aug[:, b * NTT + kt, hh, :],
                                    rhs=p_t[:, kt, qc * 512:(qc + 1) * 512],
                                    start=(kt == 0), stop=(kt == NTT - 1))

                        # normalize: inv = 1/sums; broadcast via PE outer prod
                        inv_row = stagep.tile([1, S], F32, tag="invrow")
                        nc.vector.reciprocal(out=inv_row[:], in_=po[64:65, :])
                        pinv = ps_inv.tile([64, S], F32, tag="pinv")
                        for qc in range(2):
                            nc.tensor.matmul(
                                pinv[:, qc * 512:(qc + 1) * 512],
                                lhsT=ones_s[:],
                                rhs=inv_row[:, qc * 512:(qc + 1) * 512],
                                start=True, stop=True)
                        inv_sb = stagep.tile([64, S], F32, tag="invsb")
                        nc.vector.tensor_copy(out=inv_sb[:], in_=pinv[:])
                        stg = stagep.tile([64, S], BF16, tag="stg")
                        nc.vector.tensor_mul(
                            out=stg[:], in0=po[0:64, :], in1=inv_sb[:])
                        nc.sync.dma_start(
                            out=cc_in.ap()[b, hh * 64:hh * 64 + 64, :], in_=stg[:])

            # ---------------- AllToAll ----------------
            nc.gpsimd.collective_compute(
                "AllToAll", ALU.bypass,
                replica_groups=[list(range(NCORES))],
                ins=[cc_in.ap()], outs=[cc_out.ap()],
            )

            # ---------------- phase C: dense + residual + LN ----------------
            with (
                tc.tile_pool(name="attn_sb", bufs=1) as attn_pool,
                tc.tile_pool(name="ps_y", bufs=4, space="PSUM") as ps_y,
                tc.tile_pool(name="ln", bufs=3) as lnp,
            ):
                dw_bf = attn_pool.tile([128, KT_D, D_MODEL], BF16, tag="dw")
                nc.gpsimd.dma_start(
                    out=dw_bf[:], in_=dw.ap().rearrange("(kt p) c -> p kt c", p=128))
                rw_bf = attn_pool.tile([128, KT_IN, D_MODEL], BF16, tag="rw")
                nc.gpsimd.dma_start(
                    out=rw_bf[:], in_=rw.ap().rearrange("(kt p) c -> p kt c", p=128))
                db_bc = attn_pool.tile([128, D_MODEL], F32, tag="dbc")
                nc.gpsimd.dma_start(out=db_bc[:], in_=_bcast_ap(db.ap(), 128))
                lng_bc = attn_pool.tile([128, D_MODEL], F32, tag="gbc")
                nc.gpsimd.dma_start(out=lng_bc[:], in_=_bcast_ap(lng.ap(), 128))
                lnb_bc = attn_pool.tile([128, D_MODEL], F32, tag="bbc")
                nc.gpsimd.dma_start(out=lnb_bc[:], in_=_bcast_ap(lnb.ap(), 128))
                attn_sb = attn_pool.tile([128, NCORES, S], BF16, tag="attn")
                for j in range(NCORES):
                    nc.sync.dma_start(out=attn_sb[:, j, :], in_=cc_out.ap()[j])

                for tt in range(NTT):
                    tsl = slice(tt * 128, (tt + 1) * 128)
                    y_sb = lnp.tile([128, D_MODEL], F32, tag="ysb")
                    for dc in range(2):
                        dsl = slice(dc * 512, (dc + 1) * 512)
                        py = ps_y.tile([128, 512], F32, tag="py")
                        for j in range(NCORES):
                            nc.tensor.matmul(
                                py[:], lhsT=attn_sb[:, j, tsl], rhs=dw_bf[:, j, dsl],
                                start=(j == 0), stop=False)
                        for kt in range(KT_IN):
                            nc.tensor.matmul(
                                py[:], lhsT=xres_bf[:, kt, tsl], rhs=rw_bf[:, kt, dsl],
                                start=False, stop=(kt == KT_IN - 1))
                        nc.vector.tensor_add(
                            out=y_sb[:, dsl], in0=py[:], in1=db_bc[:, dsl])

                    # layernorm over D_MODEL (1024 = 2 bn_stats subgroups)
                    stats = lnp.tile([128, 2, 6], F32, tag="stats")
                    yv = y_sb[:].rearrange("p (g d) -> p g d", g=2)
                    for g in range(2):
                        nc.vector.bn_stats(out=stats[:, g, :], in_=yv[:, g, :])
                    mv = lnp.tile([128, 2], F32, tag="mv")
                    nc.vector.bn_aggr(out=mv[:], in_=stats[:])
                    std = lnp.tile([128, 1], F32, tag="std")
                    nc.scalar.activation(out=std[:], in_=mv[:, 1:2],
                                         func=AF.Sqrt, bias=eps_sb[:], scale=1.0)
                    rstd = lnp.tile([128, 1], F32, tag="rstd")
                    nc.vector.reciprocal(out=rstd[:], in_=std[:])
                    t1 = lnp.tile([128, D_MODEL], F32, tag="t1")
                    nc.vector.scalar_tensor_tensor(
                        out=t1[:], in0=y_sb[:], scalar=mv[:, 0:1], in1=lng_bc[:],
                        op0=ALU.subtract, op1=ALU.mult)
                    yout = lnp.tile([128, D_MODEL], F32, tag="yout")
                    nc.vector.scalar_tensor_tensor(
                        out=yout[:], in0=t1[:], scalar=rstd[:], in1=lnb_bc[:],
                        op0=ALU.mult, op1=ALU.add)
                    nc.sync.dma_start(out=out.ap()[tsl, :], in_=yout[:])

    _split_multi_waits(nc)
    return nc


# ---------------------------------------------------------------------------
# Host entry point
# ---------------------------------------------------------------------------
def _make_in_maps(x, mask, wq_w, wq_b, wk_w, wk_b, wv_w, wv_b,
                  dense_w, dense_b, res_w, ln_g, ln_b):
    x = np.ascontiguousarray(np.asarray(x, dtype=np.float32).reshape(TOK, IN_DIM))
    xT = np.ascontiguousarray(x.T)
    mask = np.asarray(mask, dtype=np.int32)
    f = lambda a: np.ascontiguousarray(np.asarray(a, dtype=np.float32))
    dense_w = f(dense_w); dense_b = f(dense_b); res_w = f(res_w)
    ln_g = f(ln_g); ln_b = f(ln_b)
    in_maps = []
    for c in range(NCORES):
        csl = slice(c * COLS, (c + 1) * COLS)
        in_maps.append({
            "xT": xT,
            "xTres": np.ascontiguousarray(xT[:, c * S:(c + 1) * S]),
            "maskT": np.ascontiguousarray(
                mask[2 * c:2 * c + 2].transpose(0, 2, 1)),
            "wq": np.ascontiguousarray(f(wq_w)[:, csl]),
            "wk": np.ascontiguousarray(f(wk_w)[:, csl]),
            "wv": np.ascontiguousarray(f(wv_w)[:, csl]),
            "qb": np.ascontiguousarray(f(wq_b)[csl]),
            "kb": np.ascontiguousarray(f(wk_b)[csl]),
            "vb": np.ascontiguousarray(f(wv_b)[csl]),
            "dw": dense_w, "db": dense_b, "rw": res_w,
            "lng": ln_g, "lnb": ln_b,
        })
    return in_maps


def kernel(x, mask, wq_w, wq_b, wk_w, wk_b, wv_w, wv_b,
           dense_w, dense_b, res_w, ln_g, ln_b):
    nc = _build()
    in_maps = _make_in_maps(x, mask, wq_w, wq_b, wk_w, wk_b, wv_w, wv_b,
                            dense_w, dense_b, res_w, ln_g, ln_b)
    res = bass_utils.run_bass_kernel_spmd(
        nc, in_maps, core_ids=list(range(NCORES)))
    return np.stack([res.results[c]["out"] for c in range(NCORES)], axis=0)
